# revision 1
# baseline (speedup 1.0000x reference)
# Multi-head attention (B=2, S=2048, D=1024, H=16, head_dim=64) with bool mask,
# sharded across 8 TRN2 NeuronCores: core c -> batch c//4, heads 4*(c%4)..4*(c%4)+3.
#
# Per-core device kernel:
#   scoresT = K @ Q^T                 (PE bf16, [128 k, 1024 q] units)
#   eviction of each psS unit to bf16 attn, split 4 ways to balance engines:
#     'A': ACT exp(scale=1/8) -> DVE mask multiply
#     'P': ACT exp(scale=1/8) -> Pool (gpsimd) mask multiply
#     'Z': one fused DVE scalar_tensor_tensor: i16 <- (psS + B') * m'[k,q],
#          bit-reinterpreted as bf16 == Schraudolph exp(s/8) with the mask
#          folded in. The mask tile holds {A'=23.125, 0}; on the A/P paths the
#          same tile is a plain multiplicative mask whose uniform A' factor
#          cancels in the softmax normalization. B' is tuned so the Z path's
#          mean scale matches the A/P paths' A'*exp(s/8) exactly.
#     'I': additive fp8e5 mask folded into PSUM on the otherwise-idle PE (a
#          DoubleRow identity matmul adds -1280 to masked entries), then a
#          mask-free ACT exp with bias ln(A').
#   AV in direct layout: out[q,d] = attnT^T @ [V|1] per 128-q chunk (PE bf16,
#   full 128 output partitions; column 64 is the softmax denominator Z).
#   normalize: DVE reciprocal + broadcast multiply, assembled in SBUF, DMA out.
#
# Host side (inside kernel()): slice per-core shards, pre-transpose Q/K per head
# ([64, S] head-dim-major, bf16), pre-bake the inverted mask transposed as
# {A', 0} bf16, reassemble the 8 per-core bf16 outputs into [B, S, D] f32.

import sys

import numpy as np

for _p in ("/opt/trn_rl_repo",):
    if _p not in sys.path:
        sys.path.insert(0, _p)

import ml_dtypes

import concourse.bass as bass  # noqa: F401  (engine types reachable via nc)
import concourse.tile as tile
from concourse import bacc, mybir
from concourse.masks import make_identity

F32 = mybir.dt.float32
BF16 = mybir.dt.bfloat16
I16 = mybir.dt.int16
FP8E5 = mybir.dt.float8e5

S = 2048          # sequence length
HD = 64           # head dim
HPC = 4           # heads per core
NCORES = 8
B = 2
H = 16
D = H * HD

# Schraudolph constants for the Z path. A' is the exact bf16 rounding of
# 128/(8*ln2); B' is tuned (float32, truncating i16 cast) so that
# E[bitcast_bf16(i16((s+B')*A'))] == A' * exp(s/8) over the score distribution.
A_PRIME = 23.125
B_PRIME = 727.746979

# Optional debug map: instruction name -> semantic label (filled when
# DEBUG_LABELS is a dict; costs nothing when None).
DEBUG_LABELS = None


def _dbg(ins, label):
    if DEBUG_LABELS is not None and ins is not None:
        try:
            DEBUG_LABELS[ins.ins.name] = label
        except AttributeError:
            pass

# Per-phase eviction path patterns (16 k-strip units per phase), alternating.
# Z = fused DVE bit-trick, A = ACT exp + DVE mask, P = ACT exp + Pool mask.
# Pool mask-multiplies are the slowest (~2.1us), and the next phase's AV
# matmuls read every strip of this phase - so P units never occupy the last
# three units of a phase (a laggy Pool TT there stalls the in-order PE).
PATTERNS = ["AZAPAZPAZPAZPZAZ", "PAZPAZPAZPAZPAZA"]
PATTERNS_SMALL = ["PAZI"]
MASK_BIAS = -1280.0  # e5m2-exact; exp((s-1280)/8) == 0 for masked entries


def build_program(s=S, reps=1, patterns=PATTERNS):
    """Build the single-core SPMD program. Returns the compiled Bacc object.

    reps>1 emits the whole body that many times in one NEFF - used to measure
    device time by wall-clock differencing."""
    nc = bacc.Bacc()

    if s < 2048 and patterns is PATTERNS:
        patterns = PATTERNS_SMALL
    KS = s // 128            # number of k strips
    QG = min(1024, s)        # q width of one eviction unit
    NQG = s // QG            # q groups ("halves" at s=2048)
    NCH = QG // 128          # AV q-chunks per group
    CPG = min(4, NCH)        # chunks per psO group

    qkT_d = nc.declare_dram_parameter("qkT", [2, HPC * HD, s], BF16, isOutput=False)
    v_d = nc.declare_dram_parameter(
        "v", [s, HPC * (HD + 1)], BF16, isOutput=False
    )
    nmT_d = nc.declare_dram_parameter("nmT", [s, s], BF16, isOutput=False)
    nm8_d = nc.declare_dram_parameter("nm8", [s, s], FP8E5, isOutput=False)
    out_d = nc.declare_dram_parameter("out", [s, HPC * HD], BF16, isOutput=True)

    # Which mask formats each (g, ks) slot needs, from the per-phase paths:
    # A/Z use the bf16 multiplicative mask, P the int16 AND-mask, I the fp8
    # additive mask. Only the needed pieces are DMA'd / kept resident.
    def slot_paths(g, ks):
        return {
            patterns[(g * HPC + h) % len(patterns)][
                ks % len(patterns[(g * HPC + h) % len(patterns)])]
            for h in range(HPC)
        }

    gks = [(g, ks) for g in range(NQG) for ks in range(KS)]
    i_pieces = sorted(t for t in gks if "I" in slot_paths(*t))
    az_pieces = {t for t in gks if slot_paths(*t) & {"A", "Z", "P"}}

    nm_view = nmT_d[:].rearrange("(ks p) q -> p ks q", p=128)
    nm8_view = nm8_d[:].rearrange("(ks p) q -> p ks q", p=128)
    v_view = v_d[:].rearrange("(ks p) c -> p ks c", p=128)
    out_view = out_d[:].rearrange("(sq p) c -> p sq c", p=128)

    with tile.TileContext(nc) as tc:
        with (
            tc.tile_pool(name="const", bufs=1) as const,
            tc.tile_pool(name="wq", bufs=1) as wq,
            tc.tile_pool(name="attn", bufs=min(2 * KS + 4, 36)) as apool,
            tc.tile_pool(name="stat", bufs=4) as spool,
            tc.tile_pool(name="oasm", bufs=1) as opool,
            tc.tile_pool(name="psS", bufs=3, space="PSUM") as psS_pool,
            tc.tile_pool(name="psO", bufs=2, space="PSUM") as psO_pool,
        ):
            # Preload the exp table (emitted before any real exp; runs while
            # the first DMAs stream).
            warm = const.tile([128, 1], F32)
            nc.vector.memset(warm, 0.0)
            nc.scalar.activation(warm, warm, mybir.ActivationFunctionType.Exp)

            # fp8e5 identity pair for the DoubleRow mask-add (tile 1 = 0).
            identf = const.tile([128, 128], F32)
            make_identity(nc, identf)
            ident8 = const.tile([128, 2, 128], FP8E5)
            nc.vector.memset(ident8, 0.0)
            nc.vector.tensor_copy(out=ident8[:, 0, :], in_=identf)
            # Per-partition bias ln(A') for the I path's exp.
            lnap = const.tile([128, 1], F32)
            nc.vector.memset(lnap, float(np.log(A_PRIME)))

            # Warm the PE clock (cost model p-state ramp) while input DMAs
            # stream: ~3us of dummy matmuls.
            zb = const.tile([128, 128], BF16)
            nc.vector.memset(zb, 0.0)
            for _ in range(24):
                wmm = psS_pool.tile([128, QG], F32, tag="psS")
                nc.tensor.matmul(
                    wmm[:, :128], lhsT=zb[0:64, :], rhs=zb[0:64, :],
                    start=True, stop=True,
                )

            def qk_src(pair):
                return qkT_d[:, 128 * pair:128 * pair + 128, :].rearrange(
                    "t p s -> p t s"
                )

            def emit_body():
                # Q^T / K^T head pairs: [128, 2, s] (head 2p on partitions
                # 0-63, head 2p+1 on 64-127; dim1: 0=Q^T, 1=K^T).
                qks = []
                for pair in range(HPC // 2):
                    qk = wq.tile([128, 2, s], BF16, tag=f"qkT{pair}")
                    qks.append(qk)
                v_sb = wq.tile([128, KS, HPC * (HD + 1)], BF16, tag="vsb")
                nm_sb = wq.tile([128, KS, s], BF16, tag="nm")
                KH = KS // 2
                # All input DMAs ride the SP HWDGE queue (SP has no compute,
                # so ring-full stalls never block a compute sequencer; gpsimd
                # dma_start is SWDGE and would burn Pool engine time). Pieces
                # are ordered by first use; phases run q-group-major, so mask
                # q-group 1 is not needed until ~halfway through the kernel.
                nm8p = {}
                for (g, ks) in i_pieces:
                    t = wq.tile([128, 2, QG], FP8E5, tag=f"nm8_{g}_{ks}",
                                name=f"nm8_{g}_{ks}")
                    nm8p[(g, ks)] = t

                def nm_piece(ks, g):
                    if (g, ks) in az_pieces:
                        nc.sync.dma_start(
                            out=nm_sb[:, ks, g * QG:(g + 1) * QG],
                            in_=nm_view[:, ks, g * QG:(g + 1) * QG],
                        )
                    if ks == max(KS - 4, 0):
                        # Both DoubleRow k-tile copies point at the same data;
                        # tile 1 is multiplied by zero weights anyway.
                        for (gg, kk) in i_pieces:
                            if gg != g:
                                continue
                            for t in range(2):
                                nc.sync.dma_start(
                                    out=nm8p[(gg, kk)][:, t, :],
                                    in_=nm8_view[:, kk, gg * QG:(gg + 1) * QG],
                                )

                # First Q/K pair split by head (partition halves) so head 0's
                # slices land in ~a quarter of the full-pair DMA time.
                nc.scalar.dma_start(
                    out=qks[0][0:HD, 0, :], in_=qk_src(0)[0:HD, 0, :]
                )
                nc.sync.dma_start(
                    out=qks[0][0:HD, 1, :], in_=qk_src(0)[0:HD, 1, :]
                )
                nc.scalar.dma_start(
                    out=qks[0][HD:, 0, :], in_=qk_src(0)[HD:, 0, :]
                )
                nc.sync.dma_start(
                    out=qks[0][HD:, 1, :], in_=qk_src(0)[HD:, 1, :]
                )
                for ks in range(KS):
                    nm_piece(ks, 0)
                nc.sync.dma_start(out=v_sb[:, :KH], in_=v_view[:, :KH])
                nc.sync.dma_start(out=v_sb[:, KH:], in_=v_view[:, KH:])
                for pair in range(1, HPC // 2):
                    nc.sync.dma_start(out=qks[pair], in_=qk_src(pair))
                for g in range(1, NQG):
                    for ks in range(KS):
                        nm_piece(ks, g)


                out_asm = opool.tile([128, KS, HPC * HD], BF16)

                # q-group-major phase order: the first HPC phases only touch
                # mask q-group 0, giving the mask DMA stream headroom.
                phases = [(h, g) for g in range(NQG) for h in range(HPC)]

                def emit_av_chunk(ph, c, av_state):
                    """AV matmuls for q-chunk c of phase ph, plus group
                    finalize (reciprocal + normalize) every CPG chunks.

                    Strips are read in eviction-completion order (Z first,
                    then A, then P): the last strips read are the ones whose
                    masks lag past the phase boundary, so the PE never waits
                    on a straggling Pool/DVE mask with work still in hand."""
                    h, g = ph
                    ats = av_state["ats"]
                    order = av_state["order"]
                    if c % CPG == 0:
                        av_state["psO"] = psO_pool.tile(
                            [128, CPG, 128], F32, tag="psO", name="psO"
                        )
                    psO = av_state["psO"]
                    for i, ks in enumerate(order):
                        _dbg(nc.tensor.matmul(
                            psO[:, c % CPG, 0:HD + 1],
                            lhsT=ats[ks][:, c * 128:(c + 1) * 128],
                            rhs=v_sb[:, ks, h * (HD + 1):(h + 1) * (HD + 1)],
                            start=(i == 0),
                            stop=(i == KS - 1),
                        ), f"AV h{h}g{g} c{c} ks{ks}")
                    fe = av_state.get("fin_every", CPG)
                    if c % fe == fe - 1:
                        fin = (h, g, c - (fe - 1), fe, psO)
                        if c == NCH - 1 and av_state.get("defer_last"):
                            av_state["deferred"] = fin
                        else:
                            emit_finalize(fin)

                def emit_finalize(fin):
                    h, g, c0, fe, psO = fin
                    qc0 = g * NCH + c0
                    rec = spool.tile([128, fe], F32, tag="rec", name="rec")
                    _dbg(nc.vector.reciprocal(
                        rec, psO[:, c0 % CPG:c0 % CPG + fe, HD]),
                         f"recip h{h}g{g} c{c0}")
                    _dbg(nc.vector.tensor_mul(
                        out_asm[:, qc0:qc0 + fe, h * HD:(h + 1) * HD],
                        psO[:, c0 % CPG:c0 % CPG + fe, 0:HD],
                        rec.to_broadcast([128, fe, HD]),
                    ), f"norm h{h}g{g} c{c0}")
                    if h == HPC - 1:
                        nc.sync.dma_start(
                            out=out_view[:, qc0:qc0 + fe, :],
                            in_=out_asm[:, qc0:qc0 + fe, :],
                        )

                prev = None  # (phase, {"ats": [...]}) awaiting AV
                pending_fin = []
                unit = 0  # global eviction-unit counter (for path pattern)
                for ph in phases:
                    h, g = ph
                    base = HD * (h % 2)
                    pair = h // 2
                    q0 = g * QG
                    ats = []
                    paths = []
                    for ks in range(KS):
                        # QK for this unit
                        pat = patterns[(unit // KS) % len(patterns)]
                        path = pat[ks % len(pat)]
                        psS = psS_pool.tile([128, QG], F32, tag="psS")
                        for qc in range(QG // 512):
                            _dbg(nc.tensor.matmul(
                                psS[:, qc * 512:(qc + 1) * 512],
                                lhsT=qks[pair][base:base + HD, 1,
                                               ks * 128:(ks + 1) * 128],
                                rhs=qks[pair][base:base + HD, 0,
                                              q0 + qc * 512:q0 + (qc + 1) * 512],
                                start=True,
                                stop=(path != "I"),
                            ), f"QK h{h}g{g} ks{ks}")
                        if path == "I":
                            # Mask-add on the PE: psS += I^T @ nm8 via an fp8
                            # DoubleRow matmul (~107ns per 512 columns).
                            for qc in range(QG // 512):
                                _dbg(nc.tensor.matmul(
                                    psS[:, qc * 512:(qc + 1) * 512],
                                    lhsT=ident8,
                                    rhs=nm8p[(g, ks)][:, :,
                                                      qc * 512:(qc + 1) * 512],
                                    start=False,
                                    stop=True,
                                    perf_mode=mybir.MatmulPerfMode.DoubleRow,
                                ), f"maskI h{h}g{g} ks{ks}")
                        # Chunks ride units 4..~12: late enough that the
                        # previous phase's last evictions have drained, early
                        # enough that attn slots recycle before phase p+2.
                        if ks == 1 and pending_fin:
                            emit_finalize(pending_fin.pop())
                        if prev is not None:
                            start = 4 if KS > 8 else 1
                            den = max(KS - start - 2, 1)
                            for c in range(NCH):
                                if min(start + c * den // NCH, KS - 1) == ks:
                                    emit_av_chunk(prev[0], c, prev[1])
                        # Eviction: psS -> masked bf16 attn tile
                        at = apool.tile([128, QG], BF16, tag="at")
                        nm_slice = nm_sb[:, ks, q0:q0 + QG]
                        unit += 1
                        if path == "I":
                            # Mask already added in PSUM; exp with bias ln(A')
                            # scales the weights to match the other paths.
                            _dbg(nc.scalar.activation(
                                at, psS, mybir.ActivationFunctionType.Exp,
                                scale=0.125, bias=lnap[:],
                            ), f"expI h{h}g{g} ks{ks}")
                        elif path == "Z":
                            _dbg(nc.vector.scalar_tensor_tensor(
                                at[:].bitcast(I16),
                                psS[:],
                                B_PRIME,
                                nm_slice,
                                mybir.AluOpType.add,
                                mybir.AluOpType.mult,
                            ), f"STT h{h}g{g} ks{ks}")
                        elif path == "A":
                            _dbg(nc.scalar.activation(
                                at, psS, mybir.ActivationFunctionType.Exp,
                                scale=0.125,
                            ), f"expA h{h}g{g} ks{ks}")
                            _dbg(nc.vector.tensor_mul(at, at, nm_slice),
                                 f"maskA h{h}g{g} ks{ks}")
                        else:
                            _dbg(nc.scalar.activation(
                                at, psS, mybir.ActivationFunctionType.Exp,
                                scale=0.125,
                            ), f"expP h{h}g{g} ks{ks}")
                            _dbg(nc.gpsimd.tensor_mul(at, at, nm_slice),
                                 f"maskP h{h}g{g} ks{ks}")
                        ats.append(at)
                        paths.append(path)
                    rank = {"I": 0, "Z": 1, "A": 2, "P": 3}
                    order = sorted(range(KS), key=lambda k: (rank[paths[k]], k))
                    if prev is not None and prev[1].get("deferred"):
                        pending_fin.append(prev[1]["deferred"])
                    prev = (ph, {"ats": ats, "order": order,
                                 "defer_last": KS > 8})
                # Tail: AV of the final phase
                prev[1]["defer_last"] = False
                for fin in pending_fin:
                    emit_finalize(fin)
                for c in range(NCH):
                    emit_av_chunk(prev[0], c, prev[1])

            for _ in range(reps):
                emit_body()
    nc.compile()
    return nc


_CACHE = {}


def _get_nc():
    if "nc" not in _CACHE:
        _CACHE["nc"] = build_program()
    return _CACHE["nc"]


def make_in_maps(q, k, v, mask, s=S):
    """Shard full inputs into 8 per-core input maps (host-side layout prep)."""
    q = np.asarray(q, dtype=np.float32)
    k = np.asarray(k, dtype=np.float32)
    v = np.asarray(v, dtype=np.float32)
    mask = np.asarray(mask)
    nh = q.shape[-1] // HD
    in_maps = []
    for c in range(NCORES):
        b, g = divmod(c, NCORES // B)
        h0 = HPC * g
        qs = q[b].reshape(s, nh, HD)[:, h0:h0 + HPC, :]      # [s, HPC, 64]
        ks_ = k[b].reshape(s, nh, HD)[:, h0:h0 + HPC, :]
        qkT = np.empty((2, HPC * HD, s), ml_dtypes.bfloat16)
        qkT[0] = qs.transpose(1, 2, 0).reshape(HPC * HD, s)
        qkT[1] = ks_.transpose(1, 2, 0).reshape(HPC * HD, s)
        vh = v[b, :, h0 * HD:(h0 + HPC) * HD].reshape(s, HPC, HD)
        vc = np.concatenate(
            [vh, np.ones((s, HPC, 1), np.float32)], axis=2
        ).reshape(s, HPC * (HD + 1)).astype(ml_dtypes.bfloat16)
        mT = mask[b].T
        nmT = (np.float32(A_PRIME) * (~mT).astype(np.float32)).astype(
            ml_dtypes.bfloat16
        )
        nm8 = (np.float32(MASK_BIAS) * mT.astype(np.float32)).astype(
            ml_dtypes.float8_e5m2
        )
        in_maps.append({"qkT": qkT, "v": vc, "nmT": nmT, "nm8": nm8})
    return in_maps


def assemble_out(results, s=S, d=D):
    out = np.empty((B, s, d), np.float32)
    for c in range(NCORES):
        b, g = divmod(c, NCORES // B)
        out[b, :, g * HPC * HD:(g + 1) * HPC * HD] = results[c]["out"]
    return out


def kernel(q, k, v, mask):
    from concourse.bass_utils import run_bass_kernel_spmd

    nc = _get_nc()
    in_maps = make_in_maps(q, k, v, mask)
    res = run_bass_kernel_spmd(nc, in_maps, list(range(NCORES))).results
    return assemble_out(res)

